# revision 1
# baseline (speedup 1.0000x reference)
"""Trainium2 Bass kernel for nn_ConvBundle_48146583388363.

Math: out[x,y,b,i,j,o] = s[b, i+x-1, j+y-1] * wsum[x,y,o]
  where s = inputs.sum(channel) (zero-padded at borders) and
  wsum = W.sum(axis=2).

Sharding: data-parallel over batch B=16 across 8 cores (2 batches/core).
W and the small structural constants are replicated.

Per-core layout: flattened per-batch spatial index f = 128*t + p
(p = SBUF partition, t = tile column). The 9 tap shifts f -> f+delta
are done with 0/1 shift-matrix matmuls on the tensor engine (plus a
column-border mask), then each output tile [128 spatial, 128 cout] is a
per-partition tensor_scalar outer product, accumulated into [128, 9216]
slabs and DMA'd out as one multi-MB transfer per (tap, batch).

Note: walrus allows only ONE sync-wait on a Matmult (it rides the
LDWEIGHTS struct), so matmul operands are grouped into single DMAs and
a dummy matmul pre-syncs the shift-matrix DMA lane on PE.
"""

import numpy as np

import concourse.bacc as bacc
import concourse.bass as bass
import concourse.mybir as mybir
from concourse import tile
from concourse.bass_utils import run_bass_kernel_spmd

F32 = mybir.dt.float32

NCORES = 8
B, H, W_, CIN = 16, 96, 96, 64
COUT = 128
BPC = B // NCORES          # batches per core = 2
SP = H * W_                # 9216 spatial positions per batch
TPB = SP // 128            # 72 tiles of 128 spatial positions
NTAP = 9
TAPS = [(x - 1, y - 1) for x in range(3) for y in range(3)]  # tap n = 3x+y


def _build_consts():
    """Structural (input-independent) constants, computed on host."""
    shift_ab = np.zeros((2 * NTAP, 128, 128), np.float32)
    for n, (dx, dy) in enumerate(TAPS):
        d = 96 * dx + dy
        if d == 0:
            continue
        for m in range(128):
            k = m + d
            if 0 <= k < 128:
                shift_ab[n, k, m] = 1.0
            elif d > 0:
                shift_ab[NTAP + n, k - 128, m] = 1.0
            else:
                shift_ab[NTAP + n, k + 128, m] = 1.0
    f = 128 * np.arange(TPB)[None, :] + np.arange(128)[:, None]  # [128, 72]
    masks = np.stack([f % 96 != 0, f % 96 != 95]).astype(np.float32)
    return shift_ab, masks


def _build_nc():
    # Bacc (not raw Bass): its finalize() runs move_matmul_waits_to_ldweights
    # + generate_event_semaphores, which split multi-waits to satisfy the
    # 1-sync-wait-per-instruction hardware constraint.
    nc = bacc.Bacc(None, target_bir_lowering=False)
    x = nc.dram_tensor("x", [BPC, SP, CIN], F32, kind="ExternalInput")
    # wc[0] = all-ones (for the colsum matmul), wc[1+n] = W tap n
    wc = nc.dram_tensor("wc", [1 + NTAP, 128, COUT], F32, kind="ExternalInput")
    ab = nc.dram_tensor("ab", [2 * NTAP, 128, 128], F32, kind="ExternalInput")
    mk = nc.dram_tensor("mk", [2, 128, TPB], F32, kind="ExternalInput")
    # y is stored (p, t, o) per (tap, batch): partition-major, so each
    # partition's 72*128 floats are one contiguous 36.9KB DRAM run and the
    # slab DMA is fully linear. Host unshard permutes (p,t)->(t,p).
    y = nc.dram_tensor("y", [NTAP, BPC, 128, TPB * COUT], F32, kind="ExternalOutput")

    with tile.TileContext(nc) as tc:
        with (
            tc.tile_pool(name="const", bufs=1) as cpool,
            tc.tile_pool(name="xin", bufs=2) as xpool,
            tc.tile_pool(name="sshift", bufs=4) as spool,
            tc.tile_pool(name="psum_w", bufs=2, space="PSUM") as pwpool,
            tc.tile_pool(name="psum_s", bufs=4, space="PSUM") as pspool,
            tc.tile_pool(name="out", bufs=6) as opool,
        ):
            # Batch loads first on the ACT HWDGE ring (critical path to the
            # first slab); consts go on the otherwise-idle SP ring. Loads are
            # chunked in t-quarters so the first reduce (and the center tap's
            # output stream) starts after ~1/4 of the load.
            NQ = 2
            qt = TPB // NQ
            # Batch 0's two halves land in parallel on both HWDGE rings (it
            # gates the first slabs); consts follow on the sync ring; batch 1
            # streams on the scalar ring.
            xts = []
            for b in range(BPC):
                xt = xpool.tile([128, TPB * CIN], F32, name=f"xt{b}", tag="xt")
                xts.append(xt)

            def _load_x(b, q, eng):
                xsrc = x[b].rearrange("(t p) c -> p t c", p=128)
                eng.dma_start(
                    out=xts[b][:, q * qt * CIN:(q + 1) * qt * CIN],
                    in_=xsrc[:, q * qt:(q + 1) * qt],
                )

            _load_x(0, 0, nc.scalar)
            _load_x(0, 1, nc.sync)

            wc_sb = cpool.tile([128, (1 + NTAP) * COUT], F32, name="wc_sb")
            nc.sync.dma_start(out=wc_sb[:], in_=wc.rearrange("n k m -> k n m"))
            ab_sb = cpool.tile([128, 2 * NTAP * 128], F32, name="ab_sb")
            nc.sync.dma_start(out=ab_sb[:], in_=ab.rearrange("n k m -> k n m"))
            mk_sb = cpool.tile([128, 2 * TPB], F32, name="mk_sb")
            nc.sync.dma_start(out=mk_sb[:], in_=mk.rearrange("n p t -> p n t"))

            _load_x(1, 0, nc.scalar)
            _load_x(1, 1, nc.scalar)

            # wsum[n] = colsum of W[n], replicated across all 128 partitions
            # via ones.T @ W (one matmul does reduce + broadcast).
            ones_ap = wc_sb[:, 0:COUT]
            wsum = []
            for n in range(NTAP):
                pw = pwpool.tile([128, COUT], F32, name=f"pw{n}", tag="pw")
                nc.tensor.matmul(
                    pw[:], lhsT=ones_ap,
                    rhs=wc_sb[:, (1 + n) * COUT:(2 + n) * COUT],
                    start=True, stop=True,
                )
                ws = cpool.tile([128, COUT], F32, name=f"wsum{n}")
                nc.scalar.copy(ws[:], pw[:])
                wsum.append(ws)

            # Dummy matmul: syncs PE against the ab DMA lane so the real
            # shift matmuls carry only the DVE (s_ext) wait.
            junk = pwpool.tile([1, 1], F32, name="junk", tag="junk")
            nc.tensor.matmul(
                junk[:], lhsT=ab_sb[:, 0:1], rhs=ab_sb[:, 0:1],
                start=True, stop=True,
            )

            # s_ext[b][:, 1+t] = s for tile t; cols 0 and TPB+1 stay zero so
            # the neighbor-tile matmul can read past either end. Reduce per
            # load-quarter so downstream work starts as chunks land.
            s_ext = []
            for b in range(BPC):
                xv = xts[b][:].rearrange("p (t c) -> p t c", c=CIN)
                se = cpool.tile([128, TPB + 2], F32, name=f"s_ext{b}")
                nc.vector.memset(se[:], 0.0)
                for q in range(NQ):
                    nc.vector.reduce_sum(
                        out=se[:, 1 + q * qt:1 + (q + 1) * qt],
                        in_=xv[:, q * qt:(q + 1) * qt],
                        axis=mybir.AxisListType.X,
                    )
                s_ext.append(se)

            # Center tap first: it depends only on the reduce, not on the
            # shift matmuls, so output DMA starts earliest.
            for n, (dx, dy) in sorted(enumerate(TAPS), key=lambda e: e[1] != (0, 0)):
                d = 96 * dx + dy
                for b in range(BPC):
                    se = s_ext[b]
                    if d == 0:
                        ssh, off = se, 1
                    else:
                        ps = pspool.tile([128, TPB], F32, name=f"ps{n}_{b}", tag="ps")
                        nc.tensor.matmul(
                            ps[:], lhsT=ab_sb[:, n * 128:(n + 1) * 128],
                            rhs=se[:, 1:TPB + 1], start=True, stop=False,
                        )
                        rhs2 = se[:, 2:TPB + 2] if d > 0 else se[:, 0:TPB]
                        nc.tensor.matmul(
                            ps[:], lhsT=ab_sb[:, (NTAP + n) * 128:(NTAP + n + 1) * 128],
                            rhs=rhs2, start=False, stop=True,
                        )
                        st = spool.tile([128, TPB], F32, name=f"ssh{n}_{b}", tag="ssh")
                        if dy != 0:
                            mc = 0 if dy == -1 else 1
                            nc.vector.tensor_mul(
                                st[:], ps[:], mk_sb[:, mc * TPB:(mc + 1) * TPB]
                            )
                        else:
                            nc.vector.tensor_copy(st[:], ps[:])
                        ssh, off = st, 0

                    for h in range(2):
                        t0, t1 = h * (TPB // 2), (h + 1) * (TPB // 2)
                        slab = opool.tile(
                            [128, (TPB // 2) * COUT], F32,
                            name=f"slab{n}_{b}_{h}", tag="slab",
                        )
                        for t in range(t0, t1):
                            dst = slab[:, (t - t0) * COUT:(t - t0 + 1) * COUT]
                            sc = ssh[:, off + t:off + t + 1]
                            if t % 3 == 2:
                                nc.scalar.mul(dst, wsum[n][:], sc)
                            else:
                                nc.vector.tensor_scalar_mul(dst, wsum[n][:], sc)
                        nc.sync.dma_start(
                            out=y[n, b][:, t0 * COUT:t1 * COUT], in_=slab[:]
                        )
    nc.finalize()
    return nc


_CACHE = {}


def _get_nc():
    if "nc" not in _CACHE:
        _CACHE["nc"] = _build_nc()
        _CACHE["consts"] = _build_consts()
    return _CACHE["nc"], _CACHE["consts"]


def _run(x_full, w_full, **kwargs):
    nc, (shift_ab, masks) = _get_nc()
    wc = np.concatenate(
        [np.ones((1, 128, COUT), np.float32), w_full.reshape(NTAP, 128, COUT)]
    )
    xr = x_full.reshape(NCORES, BPC, SP, CIN)
    in_maps = [
        {
            "x": np.ascontiguousarray(xr[c]),
            "wc": wc,
            "ab": shift_ab,
            "mk": masks,
        }
        for c in range(NCORES)
    ]
    return run_bass_kernel_spmd(nc, in_maps, core_ids=list(range(NCORES)), **kwargs)


def _unshard(results):
    """Per-core y is [9, BPC, 128(p), 72(t)*128(o)]; spatial index is
    f = 128*t + p, so permute (p,t)->(t,p) while gathering."""
    out = np.empty((3, 3, B, H, W_, COUT), np.float32)
    ov = out.reshape(NTAP, B, TPB, 128, COUT)
    for c, r in enumerate(results):
        yc = r["y"].reshape(NTAP, BPC, 128, TPB, COUT)
        ov[:, BPC * c:BPC * (c + 1)] = yc.transpose(0, 1, 3, 2, 4)
    return out


def kernel(**inputs):
    x_full = np.ascontiguousarray(np.asarray(inputs["inputs"], dtype=np.float32))
    w_full = np.ascontiguousarray(np.asarray(inputs["W"], dtype=np.float32))
    res = _run(x_full, w_full)
    return _unshard(res.results)



# revision 3
# speedup vs baseline: 1.8417x; 1.8417x over previous
"""Trainium2 Bass kernel for nn_ConvBundle_48146583388363.

Math: out[x,y,b,i,j,o] = s[b, i+x-1, j+y-1] * wsum[x,y,o]
  where s = inputs.sum(channel) (zero-padded at borders) and
  wsum = W.sum(axis=2).

Sharding: data-parallel over batch B=16 across 8 cores (2 batches/core).

Layout: cout (o=128) on SBUF partitions, flat per-batch spatial index
f = 96*i + j on the free dim. One PE matmul (ones[64,128].T @ x[64,f])
does the channel reduce AND broadcasts s[f] to all 128 partitions.
Each tap's shift is then just a free-dim AP offset into an s buffer
with 98-wide zero halos; each (tap, batch, half) output is a single
[128, 4608] tensor_scalar_mul with the per-partition scalar wsum[o],
which hits the DVE 4x perf mode (fp16, step-1, 4B-aligned, SBUF).

Two s copies at both alignment parities (s_e[98+f]=s[f] on DVE,
s_o[99+f]=s[f] on ACT, both cast f32->f16 from PSUM) make every tap
offset even so the 4x mode alignment requirement holds for all 9 taps.
Column-border zeros for the dy!=0 taps are strided memsets on the
output slab before DMA.

Output is written fp16 (halves the HBM write traffic, which is the
roofline: ~45 MB/core at ~358 GB/s). Host upcasts to f32 during
unshard; rel err ~1e-3 vs the 2e-2 gate.
"""

import ml_dtypes
import numpy as np

import concourse.bacc as bacc
import concourse.mybir as mybir
from concourse import tile
from concourse.bass_utils import run_bass_kernel_spmd

F32 = mybir.dt.float32
F16 = mybir.dt.float16
BF16 = mybir.dt.bfloat16

NCORES = 8
B, H, W_, CIN = 16, 96, 96, 64
COUT = 128
BPC = B // NCORES          # batches per core = 2
SP = H * W_                # 9216 spatial positions per batch
NTAP = 9
TAPS = [(x - 1, y - 1) for x in range(3) for y in range(3)]  # tap n = 3x+y
CK = 512                   # PSUM chunk (one bank) in f
NCK = SP // CK             # 18 chunks per batch
HALO = 98
L = HALO + SP + HALO       # s buffer length = 9412
HB = SP // 2               # money-op granularity = 4608


def _build_nc():
    nc = bacc.Bacc(None, target_bir_lowering=False)
    x = nc.dram_tensor("x", [BPC, CIN, SP], BF16, kind="ExternalInput")
    w = nc.dram_tensor("w", [COUT, NTAP * COUT], F32, kind="ExternalInput")
    y = nc.dram_tensor("y", [NTAP, BPC, COUT, SP], F16, kind="ExternalOutput")

    with tile.TileContext(nc) as tc:
        with (
            tc.tile_pool(name="const", bufs=1) as cpool,
            tc.tile_pool(name="xin", bufs=1) as xpool,
            tc.tile_pool(name="psum_s", bufs=6, space="PSUM") as pspool,
            tc.tile_pool(name="psum_w", bufs=1, space="PSUM") as pwpool,
            tc.tile_pool(name="out", bufs=6) as opool,
        ):
            # --- input DMAs; batch 0 quartered for an early pipeline start
            xts = [xpool.tile([CIN, SP], BF16, name=f"xt{b}") for b in range(BPC)]
            QW = SP // 4
            for q, eng in enumerate([nc.sync, nc.scalar, nc.sync, nc.scalar]):
                eng.dma_start(
                    out=xts[0][:, q * QW:(q + 1) * QW], in_=x[0][:, q * QW:(q + 1) * QW]
                )
            w_sb = cpool.tile([COUT, NTAP * COUT], F32, name="w_sb")
            nc.scalar.dma_start(out=w_sb[:], in_=w[:])
            for hlf in range(2):
                nc.sync.dma_start(
                    out=xts[1][:, hlf * HB:(hlf + 1) * HB],
                    in_=x[1][:, hlf * HB:(hlf + 1) * HB],
                )

            # --- constants (no DMA: generated on-chip)
            ones64 = cpool.tile([CIN, COUT], BF16, name="ones64")
            nc.vector.memset(ones64[:], 1.0)
            onesc = cpool.tile([COUT, 1], F32, name="onesc")
            nc.vector.memset(onesc[:], 1.0)

            # s buffers, both parities, zero halos
            s_e, s_o = [], []
            for b in range(BPC):
                se = cpool.tile([COUT, L], F16, name=f"s_e{b}")
                so = cpool.tile([COUT, L], F16, name=f"s_o{b}")
                nc.vector.memset(se[:, 0:HALO], 0.0)
                nc.vector.memset(se[:, HALO + SP:L], 0.0)
                nc.vector.memset(so[:, 0:HALO + 1], 0.0)
                nc.vector.memset(so[:, HALO + 1 + SP:L], 0.0)
                s_e.append(se)
                s_o.append(so)

            # --- wsum[o, n] = sum_c W[n, c, o]: 9 single-column matmuls
            pw = pwpool.tile([COUT, CK], F32, name="pw")
            for n in range(NTAP):
                nc.tensor.matmul(
                    pw[:, n:n + 1],
                    lhsT=w_sb[:, n * COUT:(n + 1) * COUT],
                    rhs=onesc[:],
                    start=True, stop=True, skip_group_check=True,
                )
            ws = cpool.tile([COUT, 16], F32, name="ws")
            nc.scalar.copy(ws[:, 0:NTAP], pw[:, 0:NTAP])

            # tap read offsets (all even by construction)
            def tap_src(b, n, hlf):
                dx, dy = TAPS[n]
                d = 96 * dx + dy
                if dy == 0:
                    buf, st = s_e[b], HALO + d
                else:
                    buf, st = s_o[b], HALO + 1 + d
                return buf[:, st + hlf * HB: st + hlf * HB + HB]

            slab_seq = []  # (n, b, hlf, slab) in emission order for DMA ring mix

            def emit_money(b, hlf):
                # dy==0 taps first: they only need s_e (DVE) chunks, which
                # land earlier than the ACT s_o chunks.
                for n in sorted(range(NTAP), key=lambda n: TAPS[n][1] != 0):
                    dx, dy = TAPS[n]
                    slab = opool.tile(
                        [COUT, HB], F16, name=f"slab{n}_{b}_{hlf}", tag="slab"
                    )
                    nc.vector.tensor_scalar_mul(
                        slab[:], tap_src(b, n, hlf), ws[:, n:n + 1]
                    )
                    if dy != 0:
                        sv = slab[:].rearrange("p (i j) -> p i j", j=96)
                        jz = 0 if dy < 0 else 95
                        nc.vector.memset(sv[:, :, jz:jz + 1], 0.0)
                    slab_seq.append((n, b, hlf, slab))
                    eng = nc.sync if len(slab_seq) % 2 else nc.scalar
                    eng.dma_start(
                        out=y[n, b][:, hlf * HB:(hlf + 1) * HB], in_=slab[:]
                    )

            # --- main pipeline
            for b in range(BPC):
                for k in range(NCK):
                    ps = pspool.tile([COUT, CK], F32, name=f"ps{b}_{k}", tag="ps")
                    nc.tensor.matmul(
                        ps[:], lhsT=ones64[:],
                        rhs=xts[b][:, k * CK:(k + 1) * CK],
                        start=True, stop=True,
                    )
                    nc.vector.tensor_copy(
                        s_e[b][:, HALO + k * CK:HALO + (k + 1) * CK], ps[:]
                    )
                    nc.scalar.copy(
                        s_o[b][:, HALO + 1 + k * CK:HALO + 1 + (k + 1) * CK], ps[:]
                    )
                    if k == 9:
                        emit_money(b, 0)
                emit_money(b, 1)

    nc.finalize()
    return nc


_CACHE = {}


def _get_nc():
    if "nc" not in _CACHE:
        _CACHE["nc"] = _build_nc()
    return _CACHE["nc"]


def _run(x_full, w_full, **kwargs):
    nc = _get_nc()
    # W[n, c, o] -> [c, n*o] so the colsum matmul's lhsT ([c, o] slices) is
    # a plain contiguous SBUF tile.
    wt = np.ascontiguousarray(
        w_full.reshape(NTAP, COUT, COUT).transpose(1, 0, 2)
    ).reshape(COUT, NTAP * COUT)
    # per core: [BPC, 9216, 64] -> [BPC, 64, 9216] bf16 (channel-major so the
    # PE ones-matmul contracts over the partition dim)
    xr = x_full.reshape(NCORES, BPC, SP, CIN)
    in_maps = [
        {
            "x": np.ascontiguousarray(
                xr[c].transpose(0, 2, 1).astype(ml_dtypes.bfloat16)
            ),
            "w": wt,
        }
        for c in range(NCORES)
    ]
    return run_bass_kernel_spmd(nc, in_maps, core_ids=list(range(NCORES)), **kwargs)


def _unshard(results):
    """Per-core y is [9, BPC, 128(o), 9216(f)]; full out wants [..., f, o]."""
    out = np.empty((3, 3, B, H, W_, COUT), np.float32)
    ov = out.reshape(NTAP, B, SP, COUT)
    for c, r in enumerate(results):
        yc = np.asarray(r["y"]).reshape(NTAP, BPC, COUT, SP)
        ov[:, BPC * c:BPC * (c + 1)] = yc.transpose(0, 1, 3, 2)
    return out


def kernel(**inputs):
    x_full = np.ascontiguousarray(np.asarray(inputs["inputs"], dtype=np.float32))
    w_full = np.ascontiguousarray(np.asarray(inputs["W"], dtype=np.float32))
    res = _run(x_full, w_full)
    return _unshard(res.results)


# revision 11
# speedup vs baseline: 1.9740x; 1.0718x over previous
"""Trainium2 Bass kernel for nn_ConvBundle_48146583388363.

Math: out[x,y,b,i,j,o] = s[b, i+x-1, j+y-1] * wsum[x,y,o]
  where s = inputs.sum(channel) (zero-padded at borders) and
  wsum = W.sum(axis=2).

Sharding: data-parallel over batch B=16 across 8 cores (2 batches/core).

Layout: cout (o=128) on SBUF partitions, flat per-batch spatial index
f = 96*i + j on the free dim. One PE matmul (ones[64,128].T @ x[64,f])
does the channel reduce AND broadcasts s[f] to all 128 partitions.
Each tap's shift is then just a free-dim AP offset into an s buffer
with 98-wide zero halos; each (tap, batch, half) output is a single
[128, 4608] tensor_scalar_mul with the per-partition scalar wsum[o],
which hits the DVE 4x perf mode (fp16, step-1, 4B-aligned, SBUF).

Two s copies at both alignment parities (s_e[98+f]=s[f] on DVE,
s_o[99+f]=s[f] on ACT, both cast f32->f16 from PSUM) make every tap
offset even so the 4x mode alignment requirement holds for all 9 taps.
Column-border zeros for the dy!=0 taps are strided memsets on the
output slab before DMA.

Output is written fp16 (halves the HBM write traffic, which is the
roofline: ~45 MB/core at ~358 GB/s). Host upcasts to f32 during
unshard; rel err ~1e-3 vs the 2e-2 gate.
"""

import ml_dtypes
import numpy as np

import concourse.bacc as bacc
import concourse.mybir as mybir
from concourse import tile
from concourse.bass_utils import run_bass_kernel_spmd

F32 = mybir.dt.float32
F16 = mybir.dt.float16
BF16 = mybir.dt.bfloat16

NCORES = 8
B, H, W_, CIN = 16, 96, 96, 64
COUT = 128
BPC = B // NCORES          # batches per core = 2
SP = H * W_                # 9216 spatial positions per batch
NTAP = 9
TAPS = [(x - 1, y - 1) for x in range(3) for y in range(3)]  # tap n = 3x+y
CK = 512                   # PSUM chunk (one bank) in f
NCK = SP // CK             # 18 chunks per batch
HALO = 98
L = HALO + SP + HALO       # s buffer length = 9412
HB = SP // 2               # money-op granularity = 4608


def _build_nc():
    nc = bacc.Bacc(None, target_bir_lowering=False)
    x = nc.dram_tensor("x", [BPC, CIN, SP], BF16, kind="ExternalInput")
    w = nc.dram_tensor("w", [COUT, NTAP * COUT], BF16, kind="ExternalInput")
    y = nc.dram_tensor("y", [NTAP, BPC, COUT, SP], F16, kind="ExternalOutput")

    with tile.TileContext(nc) as tc:
        with (
            tc.tile_pool(name="const", bufs=1) as cpool,
            tc.tile_pool(name="xin", bufs=1) as xpool,
            tc.tile_pool(name="psum_s", bufs=7, space="PSUM") as pspool,
            tc.tile_pool(name="psum_w", bufs=1, space="PSUM") as pwpool,
            tc.tile_pool(name="out", bufs=5) as opool,
        ):
            # --- input DMAs. W first (it gates wsum -> every money op);
            # batch 0 quartered across both HWDGE rings for an early start.
            w_sb = cpool.tile([COUT, NTAP * COUT], BF16, name="w_sb")
            nc.scalar.dma_start(out=w_sb[:], in_=w[:])
            xts = [xpool.tile([CIN, SP], BF16, name=f"xt{b}") for b in range(BPC)]
            QW = SP // 4
            for q, eng in enumerate([nc.sync, nc.scalar, nc.sync, nc.scalar]):
                eng.dma_start(
                    out=xts[0][:, q * QW:(q + 1) * QW], in_=x[0][:, q * QW:(q + 1) * QW]
                )
            for hlf in range(2):
                nc.sync.dma_start(
                    out=xts[1][:, hlf * HB:(hlf + 1) * HB],
                    in_=x[1][:, hlf * HB:(hlf + 1) * HB],
                )

            # --- constants (no DMA: generated on-chip)
            ones64 = cpool.tile([CIN, COUT], BF16, name="ones64")
            nc.vector.memset(ones64[:], 1.0)
            onesc = cpool.tile([COUT, 1], BF16, name="onesc")
            nc.vector.memset(onesc[:], 1.0)

            # s buffers, both parities, zero halos
            s_e, s_o = [], []
            for b in range(BPC):
                se = cpool.tile([COUT, L], F16, name=f"s_e{b}")
                so = cpool.tile([COUT, L], F16, name=f"s_o{b}")
                nc.vector.memset(se[:, 0:HALO], 0.0)
                nc.vector.memset(se[:, HALO + SP:L], 0.0)
                nc.vector.memset(so[:, 0:HALO + 1], 0.0)
                nc.vector.memset(so[:, HALO + 1 + SP:L], 0.0)
                s_e.append(se)
                s_o.append(so)

            # --- wsum[o, n] = sum_c W[n, c, o]: 9 single-column matmuls.
            # ws lands via DVE (not ACT) so the s_o copy stream is never
            # head-of-line blocked behind the W dependency chain.
            pw = pwpool.tile([COUT, CK], F32, name="pw")
            for n in range(NTAP):
                nc.tensor.matmul(
                    pw[:, n:n + 1],
                    lhsT=w_sb[:, n * COUT:(n + 1) * COUT],
                    rhs=onesc[:],
                    start=True, stop=True, skip_group_check=True,
                )
            ws = cpool.tile([COUT, 16], F32, name="ws")
            nc.vector.tensor_copy(ws[:, 0:NTAP], pw[:, 0:NTAP])

            # tap read offsets (all even by construction)
            def tap_src(b, n, f0, fd):
                dx, dy = TAPS[n]
                d = 96 * dx + dy
                if dy == 0:
                    buf, st = s_e[b], HALO + d
                else:
                    buf, st = s_o[b], HALO + 1 + d
                return buf[:, st + f0: st + f0 + fd]

            slab_seq = []  # emission counter for DMA ring alternation

            def emit_money(b, f0, fd):
                # dy==0 taps first: they only need s_e (DVE) chunks, which
                # land earlier than the ACT s_o chunks.
                for n in sorted(range(NTAP), key=lambda n: TAPS[n][1] != 0):
                    dx, dy = TAPS[n]
                    slab = opool.tile(
                        [COUT, fd], F16, name=f"slab{n}_{b}_{f0}",
                        tag=f"slab{fd}",
                    )
                    nc.vector.tensor_scalar_mul(
                        slab[:], tap_src(b, n, f0, fd), ws[:, n:n + 1]
                    )
                    if dy != 0:
                        sv = slab[:].rearrange("p (i j) -> p i j", j=96)
                        jz = 0 if dy < 0 else 95
                        nc.vector.memset(sv[:, :, jz:jz + 1], 0.0)
                    slab_seq.append(n)
                    eng = nc.sync if len(slab_seq) % 2 else nc.scalar
                    eng.dma_start(
                        out=y[n, b][:, f0:f0 + fd], in_=slab[:]
                    )

            # --- main pipeline
            for b in range(BPC):
                for k in range(NCK):
                    ps = pspool.tile([COUT, CK], F32, name=f"ps{b}_{k}", tag="ps")
                    nc.tensor.matmul(
                        ps[:], lhsT=ones64[:],
                        rhs=xts[b][:, k * CK:(k + 1) * CK],
                        start=True, stop=True,
                    )
                    nc.vector.tensor_copy(
                        s_e[b][:, HALO + k * CK:HALO + (k + 1) * CK], ps[:]
                    )
                    nc.scalar.copy(
                        s_o[b][:, HALO + 1 + k * CK:HALO + 1 + (k + 1) * CK], ps[:]
                    )
                    # each piece is emitted only after every cast it reads
                    # is already in the DVE stream (no reliance on the
                    # scheduler hoisting casts past blocked money ops)
                    if b == 0 and k == 5:
                        emit_money(b, 0, HB // 2)
                    elif b == 0 and k == 9:
                        emit_money(b, HB // 2, HB // 2)
                    elif b > 0 and k == 9:
                        emit_money(b, 0, HB)
                emit_money(b, HB, HB)

    nc.finalize()
    return nc


_CACHE = {}


def _get_nc():
    if "nc" not in _CACHE:
        _CACHE["nc"] = _build_nc()
    return _CACHE["nc"]


def _run(x_full, w_full, **kwargs):
    nc = _get_nc()
    # W[n, c, o] -> [c, n*o] so the colsum matmul's lhsT ([c, o] slices) is
    # a plain contiguous SBUF tile.
    wt = np.ascontiguousarray(
        w_full.reshape(NTAP, COUT, COUT).transpose(1, 0, 2)
    ).reshape(COUT, NTAP * COUT).astype(ml_dtypes.bfloat16)
    # per core: [BPC, 9216, 64] -> [BPC, 64, 9216] bf16 (channel-major so the
    # PE ones-matmul contracts over the partition dim)
    xr = x_full.reshape(NCORES, BPC, SP, CIN)
    in_maps = [
        {
            "x": np.ascontiguousarray(
                xr[c].transpose(0, 2, 1).astype(ml_dtypes.bfloat16)
            ),
            "w": wt,
        }
        for c in range(NCORES)
    ]
    return run_bass_kernel_spmd(nc, in_maps, core_ids=list(range(NCORES)), **kwargs)


def _unshard(results):
    """Per-core y is [9, BPC, 128(o), 9216(f)]; full out wants [..., f, o]."""
    out = np.empty((3, 3, B, H, W_, COUT), np.float32)
    ov = out.reshape(NTAP, B, SP, COUT)
    for c, r in enumerate(results):
        yc = np.asarray(r["y"]).reshape(NTAP, BPC, COUT, SP)
        ov[:, BPC * c:BPC * (c + 1)] = yc.transpose(0, 1, 3, 2)
    return out


def kernel(**inputs):
    x_full = np.ascontiguousarray(np.asarray(inputs["inputs"], dtype=np.float32))
    w_full = np.ascontiguousarray(np.asarray(inputs["W"], dtype=np.float32))
    res = _run(x_full, w_full)
    return _unshard(res.results)
